# revision 2
# baseline (speedup 1.0000x reference)
"""GRU cell kernel for Trainium2, data-parallel over batch across 8 NeuronCores.

Reference computation (B=8192, D=H=1024), per batch row:
    z = sigmoid(inp@wz + state@uz + bz)
    r = sigmoid(inp@wr + state@ur + br)
    h_ = tanh(inp@wx + bx + (state@wh) * r)
    hid = (1-z)*h_ + state*z

Strategy: each core takes a 1024-row batch shard. The z/r projections fuse
into one [1024,2048]@[2048,2048] GEMM (act = [inp|state], W = [[wz,wr],[uz,ur]]).
xh and hh stay separate GEMMs ([1024,1024]@[1024,1024] each) because hh is
gated by r before the sum. Activations are shipped pre-transposed from the
host ([K,B] layout) so they can be the stationary matmul operand; weights
stream from HBM as the moving operand. Biases enter the PSUM accumulation
group as a K=1 rank-one matmul against a ones-row. A fused DVE/ACT epilogue
reads PSUM and writes the output shard.
"""

import os
import sys
import types

sys.path.insert(0, "/opt/trn_rl_repo")

import numpy as np

# trace=True under axon needs antenv.axon_hooks, absent from this image.
# Register the same ctypes-backed NTFF hook trn_boot would have installed.
if "antenv.axon_hooks" not in sys.modules:
    _m = types.ModuleType("antenv.axon_hooks")
    _m._hook = None

    def _set_hook(h):
        _m._hook = h

    def _get_hook():
        return _m._hook

    _m.set_axon_ntff_profile_hook = _set_hook
    _m.get_axon_ntff_profile_hook = _get_hook
    sys.modules["antenv.axon_hooks"] = _m
    try:
        from trn_agent_boot.trn_boot import _ntff_profile_via_ctypes

        _m.set_axon_ntff_profile_hook(
            _ntff_profile_via_ctypes("/opt/axon/libaxon_pjrt.so")
        )
    except Exception:
        pass

import concourse.bacc as bacc
import concourse.tile as tile
from concourse import mybir
from concourse.bass_utils import run_bass_kernel_spmd

N_CORES = 8
B, D, H = 8192, 1024, 1024
BL = B // N_CORES  # batch rows per core
P = 128  # partitions
NF = 512  # matmul free dim (one PSUM bank of fp32)
KD = D // P  # k-tiles per 1024 contraction
MT = BL // P  # batch m-tiles per core
F32 = mybir.dt.float32

_CACHE = {}


def _build_program():
    nc = bacc.Bacc("TRN2", target_bir_lowering=False, debug=False)

    xT = nc.declare_dram_parameter("xT", [D, BL], F32, isOutput=False)
    sT = nc.declare_dram_parameter("sT", [H, BL], F32, isOutput=False)
    st = nc.declare_dram_parameter("st", [BL, H], F32, isOutput=False)
    wzr = nc.declare_dram_parameter("wzr", [D + H, 2 * H], F32, isOutput=False)
    wx = nc.declare_dram_parameter("wx", [D, H], F32, isOutput=False)
    wh = nc.declare_dram_parameter("wh", [H, H], F32, isOutput=False)
    bzr = nc.declare_dram_parameter("bzr", [1, 2 * H], F32, isOutput=False)
    bx = nc.declare_dram_parameter("bx", [1, H], F32, isOutput=False)
    out = nc.declare_dram_parameter("out", [BL, H], F32, isOutput=True)

    with tile.TileContext(nc) as tc:
        with (
            tc.tile_pool(name="acts", bufs=1) as acts,
            tc.tile_pool(name="stash", bufs=1) as stash,
            tc.tile_pool(name="wpool", bufs=18) as wpool,
            tc.tile_pool(name="stp", bufs=4) as stp,
            tc.tile_pool(name="tmp", bufs=2) as tmp,
            tc.tile_pool(name="small", bufs=1) as small,
            tc.tile_pool(name="ps", bufs=8, space="PSUM") as ps,
        ):
            ones = small.tile([1, P], F32, tag="ones")
            nc.vector.memset(ones, 1.0)
            bzr_sb = small.tile([1, 2 * H], F32, tag="bzr")
            nc.sync.dma_start(out=bzr_sb, in_=bzr.ap())
            bx_sb = small.tile([1, H], F32, tag="bx")
            nc.sync.dma_start(out=bx_sb, in_=bx.ap())

            # Resident transposed activations: [K, B_local] k-tiles.
            xT_t, sT_t = [], []
            for k in range(KD):
                t = acts.tile([P, BL], F32, tag=f"xT{k}", name=f"xT{k}")
                nc.sync.dma_start(out=t, in_=xT.ap()[k * P : (k + 1) * P, :])
                xT_t.append(t)
            for k in range(KD):
                t = acts.tile([P, BL], F32, tag=f"sT{k}", name=f"sT{k}")
                nc.sync.dma_start(out=t, in_=sT.ap()[k * P : (k + 1) * P, :])
                sT_t.append(t)

            # Sigmoid outputs stashed until the final gate pass.
            z_st = [stash.tile([P, H], F32, tag=f"z{m}", name=f"z{m}") for m in range(MT)]
            r_st = [stash.tile([P, H], F32, tag=f"r{m}", name=f"r{m}") for m in range(MT)]

            # ---- Phase 1: fused z/r GEMM, K=2048, N=2048 ----
            for g in range(4):  # 512-wide column block of the 2048 zr space
                wt = []
                for k in range(2 * KD):
                    w = wpool.tile([P, NF], F32, tag="w", name="w")
                    nc.sync.dma_start(
                        out=w,
                        in_=wzr.ap()[
                            k * P : (k + 1) * P, g * NF : (g + 1) * NF
                        ],
                    )
                    wt.append(w)
                for m in range(MT):
                    acc = ps.tile([P, NF], F32, tag="ps", name="acc")
                    msl = slice(m * P, (m + 1) * P)
                    nc.tensor.matmul(
                        acc,
                        ones,
                        bzr_sb[:, g * NF : (g + 1) * NF],
                        start=True,
                        stop=False,
                    )
                    for k in range(2 * KD):
                        lhsT = (xT_t[k] if k < KD else sT_t[k - KD])[:, msl]
                        nc.tensor.matmul(
                            acc, lhsT, wt[k], start=False, stop=(k == 2 * KD - 1)
                        )
                    dst = (z_st if g < 2 else r_st)[m][
                        :, (g % 2) * NF : (g % 2 + 1) * NF
                    ]
                    nc.scalar.activation(
                        dst, acc, mybir.ActivationFunctionType.Sigmoid
                    )

            # ---- Phase 2: xh & hh GEMMs + fused gate epilogue ----
            for c in range(2):  # 512-wide column block of H
                csl = slice(c * NF, (c + 1) * NF)
                wxt, wht = [], []
                for k in range(KD):
                    w = wpool.tile([P, NF], F32, tag="w", name="w")
                    nc.sync.dma_start(
                        out=w, in_=wx.ap()[k * P : (k + 1) * P, csl]
                    )
                    wxt.append(w)
                for k in range(KD):
                    w = wpool.tile([P, NF], F32, tag="w", name="w")
                    nc.sync.dma_start(
                        out=w, in_=wh.ap()[k * P : (k + 1) * P, csl]
                    )
                    wht.append(w)
                for m in range(MT):
                    msl = slice(m * P, (m + 1) * P)
                    st_t = stp.tile([P, NF], F32, tag="st", name="st_t")
                    nc.sync.dma_start(out=st_t, in_=st.ap()[msl, csl])

                    pxh = ps.tile([P, NF], F32, tag="ps", name="pxh")
                    nc.tensor.matmul(
                        pxh, ones, bx_sb[:, csl], start=True, stop=False
                    )
                    for k in range(KD):
                        nc.tensor.matmul(
                            pxh,
                            xT_t[k][:, msl],
                            wxt[k],
                            start=False,
                            stop=(k == KD - 1),
                        )
                    phh = ps.tile([P, NF], F32, tag="ps", name="phh")
                    for k in range(KD):
                        nc.tensor.matmul(
                            phh,
                            sT_t[k][:, msl],
                            wht[k],
                            start=(k == 0),
                            stop=(k == KD - 1),
                        )

                    # h_ = tanh(xh + hh*r); hid = h_ + z*(state - h_)
                    t = tmp.tile([P, NF], F32, tag="t", name="t")
                    nc.vector.tensor_mul(t, phh, r_st[m][:, csl])
                    nc.vector.tensor_add(t, t, pxh)
                    h = tmp.tile([P, NF], F32, tag="h", name="h")
                    nc.scalar.activation(h, t, mybir.ActivationFunctionType.Tanh)
                    d = tmp.tile([P, NF], F32, tag="d", name="d")
                    nc.vector.tensor_sub(d, st_t, h)
                    nc.vector.tensor_mul(d, d, z_st[m][:, csl])
                    o = tmp.tile([P, NF], F32, tag="o", name="o")
                    nc.vector.tensor_add(o, h, d)
                    nc.sync.dma_start(out=out.ap()[msl, csl], in_=o)

    nc.compile()
    return nc


def _get_program():
    if "nc" not in _CACHE:
        _CACHE["nc"] = _build_program()
    return _CACHE["nc"]


def kernel(inp, state, wx, bx, wh, wr, ur, uz, wz, br, bz):
    inp = np.asarray(inp, dtype=np.float32)
    state = np.asarray(state, dtype=np.float32)
    w_zr = np.block(
        [
            [np.asarray(wz, np.float32), np.asarray(wr, np.float32)],
            [np.asarray(uz, np.float32), np.asarray(ur, np.float32)],
        ]
    )
    w_x = np.ascontiguousarray(np.asarray(wx, np.float32))
    w_h = np.ascontiguousarray(np.asarray(wh, np.float32))
    b_zr = np.concatenate(
        [np.asarray(bz, np.float32), np.asarray(br, np.float32)]
    )[None, :]
    b_x = np.ascontiguousarray(np.asarray(bx, np.float32))[None, :]
    xT = np.ascontiguousarray(inp.T)
    sT = np.ascontiguousarray(state.T)

    in_maps = []
    for c in range(N_CORES):
        sl = slice(c * BL, (c + 1) * BL)
        in_maps.append(
            {
                "xT": np.ascontiguousarray(xT[:, sl]),
                "sT": np.ascontiguousarray(sT[:, sl]),
                "st": np.ascontiguousarray(state[sl]),
                "wzr": w_zr,
                "wx": w_x,
                "wh": w_h,
                "bzr": b_zr,
                "bx": b_x,
            }
        )

    nc = _get_program()
    trace = bool(int(os.environ.get("GRU_TRACE", "0")))
    res = run_bass_kernel_spmd(nc, in_maps, list(range(N_CORES)), trace=trace)
    if trace:
        _CACHE["last_exec_time_ns"] = res.exec_time_ns
        _CACHE["last_results"] = res
    return np.concatenate([res.results[c]["out"] for c in range(N_CORES)], axis=0)


# revision 3
# speedup vs baseline: 3.2096x; 3.2096x over previous
"""GRU cell kernel for Trainium2, data-parallel over batch across 8 NeuronCores.

Reference computation (B=8192, D=H=1024), per batch row:
    z = sigmoid(inp@wz + state@uz + bz)
    r = sigmoid(inp@wr + state@ur + br)
    h_ = tanh(inp@wx + bx + (state@wh) * r)
    hid = (1-z)*h_ + state*z

Strategy: each core takes a 1024-row batch shard. The z/r projections fuse
into one [1024,2048]@[2048,2048] GEMM (act = [inp|state], W = [[wz,wr],[uz,ur]]).
xh and hh stay separate GEMMs ([1024,1024]@[1024,1024] each) because hh is
gated by r before the sum. Activations are shipped pre-transposed from the
host ([K,B] layout) so they can be the stationary matmul operand; weights
stream from HBM as the moving operand. Biases enter the PSUM accumulation
group as a K=1 rank-one matmul against a ones-row. A fused DVE/ACT epilogue
reads PSUM and writes the output shard.
"""

import os
import sys
import types

sys.path.insert(0, "/opt/trn_rl_repo")

import numpy as np

# trace=True under axon needs antenv.axon_hooks, absent from this image.
# Register the same ctypes-backed NTFF hook trn_boot would have installed.
if "antenv.axon_hooks" not in sys.modules:
    _m = types.ModuleType("antenv.axon_hooks")
    _m._hook = None

    def _set_hook(h):
        _m._hook = h

    def _get_hook():
        return _m._hook

    _m.set_axon_ntff_profile_hook = _set_hook
    _m.get_axon_ntff_profile_hook = _get_hook
    sys.modules["antenv.axon_hooks"] = _m
    try:
        from trn_agent_boot.trn_boot import _ntff_profile_via_ctypes

        _m.set_axon_ntff_profile_hook(
            _ntff_profile_via_ctypes("/opt/axon/libaxon_pjrt.so")
        )
    except Exception:
        pass

import concourse.bacc as bacc
import concourse.tile as tile
from concourse import mybir
from concourse.bass_utils import run_bass_kernel_spmd

N_CORES = 8
B, D, H = 8192, 1024, 1024
BL = B // N_CORES  # batch rows per core
P = 128  # partitions
NF = 512  # matmul free dim (one PSUM bank of fp32)
KD = D // P  # k-tiles per 1024 contraction
MT = BL // P  # batch m-tiles per core
F32 = mybir.dt.float32
F32R = mybir.dt.float32r

_CACHE = {}


def _build_program(with_bias):
    nc = bacc.Bacc("TRN2", target_bir_lowering=False, debug=False)

    xT = nc.declare_dram_parameter("xT", [D, BL], F32R, isOutput=False)
    sT = nc.declare_dram_parameter("sT", [H, BL], F32R, isOutput=False)
    st = nc.declare_dram_parameter("st", [BL, H], F32, isOutput=False)
    wzr = nc.declare_dram_parameter("wzr", [D + H, 2 * H], F32R, isOutput=False)
    wx = nc.declare_dram_parameter("wx", [D, H], F32R, isOutput=False)
    wh = nc.declare_dram_parameter("wh", [H, H], F32R, isOutput=False)
    if with_bias:
        bzr = nc.declare_dram_parameter("bzr", [1, 2 * H], F32R, isOutput=False)
        bx = nc.declare_dram_parameter("bx", [1, H], F32R, isOutput=False)
    out = nc.declare_dram_parameter("out", [BL, H], F32, isOutput=True)

    with tile.TileContext(nc) as tc:
        with (
            tc.tile_pool(name="acts", bufs=1) as acts,
            tc.tile_pool(name="stash", bufs=1) as stash,
            tc.tile_pool(name="wpool", bufs=18) as wpool,
            tc.tile_pool(name="stp", bufs=4) as stp,
            tc.tile_pool(name="tmp", bufs=2) as tmp,
            tc.tile_pool(name="small", bufs=1) as small,
            tc.tile_pool(name="ps", bufs=8, space="PSUM") as ps,
        ):
            if with_bias:
                ones = small.tile([1, P], F32R, tag="ones")
                nc.vector.memset(ones, 1.0)
                bzr_sb = small.tile([1, 2 * H], F32R, tag="bzr")
                nc.sync.dma_start(out=bzr_sb, in_=bzr.ap())
                bx_sb = small.tile([1, H], F32R, tag="bx")
                nc.sync.dma_start(out=bx_sb, in_=bx.ap())

            # Resident transposed activations: [K, B_local] k-tiles.
            xT_t, sT_t = [], []
            for k in range(KD):
                t = acts.tile([P, BL], F32R, tag=f"xT{k}", name=f"xT{k}")
                nc.sync.dma_start(out=t, in_=xT.ap()[k * P : (k + 1) * P, :])
                xT_t.append(t)
            for k in range(KD):
                t = acts.tile([P, BL], F32R, tag=f"sT{k}", name=f"sT{k}")
                nc.sync.dma_start(out=t, in_=sT.ap()[k * P : (k + 1) * P, :])
                sT_t.append(t)

            # Sigmoid outputs stashed until the final gate pass.
            z_st = [stash.tile([P, H], F32, tag=f"z{m}", name=f"z{m}") for m in range(MT)]
            r_st = [stash.tile([P, H], F32, tag=f"r{m}", name=f"r{m}") for m in range(MT)]

            # ---- Phase 1: fused z/r GEMM, K=2048, N=2048 ----
            for g in range(4):  # 512-wide column block of the 2048 zr space
                wt = []
                for k in range(2 * KD):
                    w = wpool.tile([P, NF], F32R, tag="w", name="w")
                    nc.sync.dma_start(
                        out=w,
                        in_=wzr.ap()[
                            k * P : (k + 1) * P, g * NF : (g + 1) * NF
                        ],
                    )
                    wt.append(w)
                for m in range(MT):
                    acc = ps.tile([P, NF], F32, tag="ps", name="acc")
                    msl = slice(m * P, (m + 1) * P)
                    if with_bias:
                        nc.tensor.matmul(
                            acc,
                            ones,
                            bzr_sb[:, g * NF : (g + 1) * NF],
                            start=True,
                            stop=False,
                        )
                    for k in range(2 * KD):
                        lhsT = (xT_t[k] if k < KD else sT_t[k - KD])[:, msl]
                        nc.tensor.matmul(
                            acc,
                            lhsT,
                            wt[k],
                            start=(k == 0 and not with_bias),
                            stop=(k == 2 * KD - 1),
                        )
                    dst = (z_st if g < 2 else r_st)[m][
                        :, (g % 2) * NF : (g % 2 + 1) * NF
                    ]
                    nc.scalar.activation(
                        dst, acc, mybir.ActivationFunctionType.Sigmoid
                    )

            # ---- Phase 2: xh & hh GEMMs + fused gate epilogue ----
            for c in range(2):  # 512-wide column block of H
                csl = slice(c * NF, (c + 1) * NF)
                wxt, wht = [], []
                for k in range(KD):
                    w = wpool.tile([P, NF], F32R, tag="w", name="w")
                    nc.sync.dma_start(
                        out=w, in_=wx.ap()[k * P : (k + 1) * P, csl]
                    )
                    wxt.append(w)
                for k in range(KD):
                    w = wpool.tile([P, NF], F32R, tag="w", name="w")
                    nc.sync.dma_start(
                        out=w, in_=wh.ap()[k * P : (k + 1) * P, csl]
                    )
                    wht.append(w)
                for m in range(MT):
                    msl = slice(m * P, (m + 1) * P)
                    st_t = stp.tile([P, NF], F32, tag="st", name="st_t")
                    nc.sync.dma_start(out=st_t, in_=st.ap()[msl, csl])

                    pxh = ps.tile([P, NF], F32, tag="ps", name="pxh")
                    if with_bias:
                        nc.tensor.matmul(
                            pxh, ones, bx_sb[:, csl], start=True, stop=False
                        )
                    for k in range(KD):
                        nc.tensor.matmul(
                            pxh,
                            xT_t[k][:, msl],
                            wxt[k],
                            start=(k == 0 and not with_bias),
                            stop=(k == KD - 1),
                        )
                    phh = ps.tile([P, NF], F32, tag="ps", name="phh")
                    for k in range(KD):
                        nc.tensor.matmul(
                            phh,
                            sT_t[k][:, msl],
                            wht[k],
                            start=(k == 0),
                            stop=(k == KD - 1),
                        )

                    # h_ = tanh(xh + hh*r); hid = h_ + z*(state - h_)
                    t = tmp.tile([P, NF], F32, tag="t", name="t")
                    nc.vector.tensor_mul(t, phh, r_st[m][:, csl])
                    nc.vector.tensor_add(t, t, pxh)
                    h = tmp.tile([P, NF], F32, tag="h", name="h")
                    nc.scalar.activation(h, t, mybir.ActivationFunctionType.Tanh)
                    d = tmp.tile([P, NF], F32, tag="d", name="d")
                    nc.vector.tensor_sub(d, st_t, h)
                    nc.vector.tensor_mul(d, d, z_st[m][:, csl])
                    o = tmp.tile([P, NF], F32, tag="o", name="o")
                    nc.vector.tensor_add(o, h, d)
                    nc.sync.dma_start(out=out.ap()[msl, csl], in_=o)

    nc.compile()
    return nc


def _get_program(with_bias):
    key = ("nc", with_bias)
    if key not in _CACHE:
        _CACHE[key] = _build_program(with_bias)
    return _CACHE[key]


def kernel(inp, state, wx, bx, wh, wr, ur, uz, wz, br, bz):
    inp = np.asarray(inp, dtype=np.float32)
    state = np.asarray(state, dtype=np.float32)
    w_zr = np.block(
        [
            [np.asarray(wz, np.float32), np.asarray(wr, np.float32)],
            [np.asarray(uz, np.float32), np.asarray(ur, np.float32)],
        ]
    )
    w_x = np.ascontiguousarray(np.asarray(wx, np.float32))
    w_h = np.ascontiguousarray(np.asarray(wh, np.float32))
    b_zr = np.concatenate(
        [np.asarray(bz, np.float32), np.asarray(br, np.float32)]
    )[None, :]
    b_x = np.ascontiguousarray(np.asarray(bx, np.float32))[None, :]
    xT = np.ascontiguousarray(inp.T)
    sT = np.ascontiguousarray(state.T)

    with_bias = bool(np.any(b_zr) or np.any(b_x))
    in_maps = []
    for c in range(N_CORES):
        sl = slice(c * BL, (c + 1) * BL)
        im = {
            "xT": np.ascontiguousarray(xT[:, sl]),
            "sT": np.ascontiguousarray(sT[:, sl]),
            "st": np.ascontiguousarray(state[sl]),
            "wzr": w_zr,
            "wx": w_x,
            "wh": w_h,
        }
        if with_bias:
            im["bzr"] = b_zr
            im["bx"] = b_x
        in_maps.append(im)

    nc = _get_program(with_bias)
    trace = bool(int(os.environ.get("GRU_TRACE", "0")))
    res = run_bass_kernel_spmd(nc, in_maps, list(range(N_CORES)), trace=trace)
    if trace:
        _CACHE["last_exec_time_ns"] = res.exec_time_ns
        _CACHE["last_results"] = res
    return np.concatenate([res.results[c]["out"] for c in range(N_CORES)], axis=0)
